# revision 24
# baseline (speedup 1.0000x reference)
"""BertAlibiLayer on 8 TRN2 NeuronCores — data-parallel over batch.

Layout strategy: all activations on-chip are FEATURE-major ([feature, token]),
which makes every matmul transpose-free (weights are pre-transposed on host).
Attention computes scoresT = [key, query]; softmax normalization comes from a
ones-column folded into V (denominator lands as a psum row) and is applied as
exp(-ln(denom)) broadcast across partitions by GPSIMD. LayerNorm reductions
(over features = partitions) use ones-vector matmuls on the PE; mean/rstd are
broadcast back across partitions by GPSIMD into SBUF.

Per core: 2 sequences x 512 tokens (N=1024 token-columns), full weights.
Projection/MLP matmuls run as float32r (full-rate fp32 streaming); the
attention probs path and the Wdown contraction use bf16.

PSUM pools (4+2+2 banks) stay open for the whole kernel so no phase ever
serializes on bank reuse.
"""

from contextlib import ExitStack

import numpy as np
import ml_dtypes

import concourse.bass as bass
import concourse.mybir as mybir
import concourse.tile as tile
from concourse import bacc
from concourse.bass_utils import run_bass_kernel_spmd

F32 = mybir.dt.float32
F32R = mybir.dt.float32r
BF16 = mybir.dt.bfloat16
AF = mybir.ActivationFunctionType
OP = mybir.AluOpType

DIM = 768
H = 12
HD = 64
S = 512
NSEQ = 2          # sequences per core
N = NSEQ * S      # tokens per core
I = 3072
KT = DIM // 128   # 6 k-tiles over DIM
EPS = 1e-12
N_CORES = 8

OC_ORDER = [0, 6, 1, 7, 2, 8, 3, 9, 4, 10, 5, 11]  # q/k chunk emission order


def r(ap):
    """View an fp32 AP as float32r for full-rate PE streaming."""
    return ap.bitcast(F32R)


def build_program(gelu_func=AF.Gelu):
    nc = bacc.Bacc("TRN2", target_bir_lowering=False, debug=False,
                   enable_asserts=False)
    # Steer the act-table chooser: the plain natural_log set lacks exp, so
    # Ln<->Exp sequences would reload tables every op. Emptying it (in place,
    # preserving set ids) makes the chooser use natural_log_exp_and_others.
    import concourse.hw_specs as hw_specs
    tabs = hw_specs.get_activation_tables(nc.m.arch)
    tabs["natural_log"] = set()

    # ---- DRAM parameters (per-core shards / replicated weights) ----
    xT = nc.dram_tensor("xT", [DIM, N], F32R, kind="ExternalInput").ap()
    # exp(bias)^T: softmax uses exp(s+b) = exp(s)*exp(b), exp(b) from host
    expbT = nc.dram_tensor("expbT", [NSEQ, H, S, S], BF16, kind="ExternalInput").ap()
    wqkvT = nc.dram_tensor("wqkvT", [DIM, 3 * DIM], F32R, kind="ExternalInput").ap()
    bqk = nc.dram_tensor("bqk", [128, 12], F32, kind="ExternalInput").ap()
    bv_b = nc.dram_tensor("bv_b", [128, DIM], F32, kind="ExternalInput").ap()
    woT = nc.dram_tensor("woT", [DIM, DIM], F32R, kind="ExternalInput").ap()
    bo = nc.dram_tensor("bo", [128, 6], F32, kind="ExternalInput").ap()
    wgluT = nc.dram_tensor("wgluT", [DIM, 2 * I], F32R, kind="ExternalInput").ap()
    cb1 = nc.dram_tensor("cb1", [128, 48], F32, kind="ExternalInput").ap()
    g1 = nc.dram_tensor("g1", [128, 6], F32, kind="ExternalInput").ap()
    c1 = nc.dram_tensor("c1", [128, 6], F32, kind="ExternalInput").ap()
    wdownT = nc.dram_tensor("wdownT", [I, DIM], BF16, kind="ExternalInput").ap()
    g2 = nc.dram_tensor("g2", [128, 6], F32, kind="ExternalInput").ap()
    b2 = nc.dram_tensor("b2", [128, 6], F32, kind="ExternalInput").ap()
    outT = nc.dram_tensor("outT", [DIM, N], F32, kind="ExternalOutput").ap()

    with tile.TileContext(nc) as tc:
        emit(nc, tc, xT, expbT, wqkvT, bqk, bv_b, woT, bo, wgluT, cb1, g1, c1,
             wdownT, g2, b2, outT, gelu_func)

    nc.compile()
    return nc


def emit(nc, tc, xT, expbT, wqkvT, bqk, bv_b, woT, bo, wgluT, cb1, g1, c1,
         wdownT, g2, b2, outT, gelu_func=AF.Gelu):
    root = ExitStack()
    consts = root.enter_context(tc.tile_pool(name="consts", bufs=1, side="left"))
    bcln_pool = root.enter_context(tc.tile_pool(name="bcln", bufs=2, side="left"))
    # PSUM pools — opened once for the whole kernel (8 banks total) so no
    # phase transition ever waits on bank reuse.
    pmm = root.enter_context(tc.tile_pool(name="pmm", bufs=4, space="PSUM"))
    pctx = root.enter_context(tc.tile_pool(name="pctx", bufs=2, space="PSUM"))
    pstat = root.enter_context(tc.tile_pool(name="pstat", bufs=2, space="PSUM"))

    # ---------------- Phase 1: QKV projection ----------------
    xt_ctx = ExitStack()
    xt_pool = xt_ctx.enter_context(tc.tile_pool(name="xt", bufs=KT, side="left"))
    qkva_ctx = ExitStack()
    qk_pool = qkva_ctx.enter_context(tc.tile_pool(name="qk", bufs=12, side="left"))
    va_pool = qkva_ctx.enter_context(tc.tile_pool(name="vaug", bufs=8, side="left"))
    p1_ctx = ExitStack()
    wq_pool = p1_ctx.enter_context(tc.tile_pool(name="wqkv", bufs=6, side="left"))

    # Critical-path DMAs first: the very first matmul chain (oc=0, half 0)
    # needs chunks (kt, 0) and xt half-0 only (~2MB), so those dispatch first.
    wqk_sb = [[None] * 12 for _ in range(KT)]
    xt_sb = []
    for kt in range(KT):
        c = wq_pool.tile([128, 128], F32R, name=f"wqk{kt}_0", tag="wqk",
                         bufs=72)
        nc.sync.dma_start(c[:], wqkvT[kt * 128:(kt + 1) * 128, 0:128])
        wqk_sb[kt][0] = c
    for kt in range(KT):
        t = xt_pool.tile([128, N], F32R, name=f"xt{kt}", tag="xt")
        nc.sync.dma_start(t[:, 0:512], xT[kt * 128:(kt + 1) * 128, 0:512])
        xt_sb.append(t)
    for kt in range(KT):
        oc = 6
        c = wq_pool.tile([128, 128], F32R, name=f"wqk{kt}_6", tag="wqk",
                         bufs=72)
        nc.sync.dma_start(c[:], wqkvT[kt * 128:(kt + 1) * 128,
                                      oc * 128:(oc + 1) * 128])
        wqk_sb[kt][6] = c

    # small constant tensors
    bqk_sb = consts.tile([128, 12], F32)
    nc.sync.dma_start(bqk_sb[:], bqk[:, :])
    bvb_sb = consts.tile([128, DIM], F32)
    nc.sync.dma_start(bvb_sb[:], bv_b[:, :])
    bo_sb = consts.tile([128, 6], F32)
    nc.sync.dma_start(bo_sb[:], bo[:, :])
    cb1_sb = consts.tile([128, 48], F32)
    nc.sync.dma_start(cb1_sb[:], cb1[:, :])
    g1_sb = consts.tile([128, 6], F32)
    nc.sync.dma_start(g1_sb[:], g1[:, :])
    c1_sb = consts.tile([128, 6], F32)
    nc.sync.dma_start(c1_sb[:], c1[:, :])
    g2_sb = consts.tile([128, 6], F32)
    nc.sync.dma_start(g2_sb[:], g2[:, :])
    b2_sb = consts.tile([128, 6], F32)
    nc.sync.dma_start(b2_sb[:], b2[:, :])
    # f32 ones staging (memset cannot write f32r; DVE copies round instead)
    ones_f32c = consts.tile([128, 12], F32)
    nc.vector.memset(ones_f32c[:], 1.0)
    ones_col = consts.tile([128, 1], F32)   # stats lhsT: column of ones
    nc.vector.tensor_copy(ones_col[:].bitcast(F32R), ones_f32c[:, 0:1])
    eps_sb = consts.tile([1, 1], F32)
    nc.vector.memset(eps_sb[:], EPS)

    # remaining x columns + weight chunks in chain-consumption order
    for kt in range(KT):
        nc.sync.dma_start(xt_sb[kt][:, 512:1024],
                          xT[kt * 128:(kt + 1) * 128, 512:1024])
    for oc in OC_ORDER[2:]:
        for kt in range(KT):
            c = wq_pool.tile([128, 128], F32R, name=f"wqk{kt}_{oc}", tag="wqk",
                             bufs=72)
            nc.sync.dma_start(c[:], wqkvT[kt * 128:(kt + 1) * 128,
                                          oc * 128:(oc + 1) * 128])
            wqk_sb[kt][oc] = c
    wv_sb = []
    for kt in range(KT):
        v = wq_pool.tile([128, DIM], F32R, name=f"wv{kt}", tag="wv", bufs=6)
        nc.sync.dma_start(v[:], wqkvT[kt * 128:(kt + 1) * 128, 2 * DIM:])
        wv_sb.append(v)

    # q,k in feature-major [feature, token]; q columns pre-scaled by 1/8 on host
    qk_sb = [None] * 12
    for oc in OC_ORDER:
        qt = qk_pool.tile([128, N], BF16, name=f"qk{oc}", tag="qk")
        qk_sb[oc] = qt
        for h2 in range(2):
            ps = pmm.tile([128, 512], F32, tag="ps")
            for kt in range(KT):
                nc.tensor.matmul(
                    ps[:], wqk_sb[kt][oc][:],
                    r(xt_sb[kt][:, h2 * 512:(h2 + 1) * 512]),
                    start=(kt == 0), stop=(kt == KT - 1),
                )
            nc.scalar.activation(qt[:, h2 * 512:(h2 + 1) * 512],
                                 ps[:], AF.Identity, bias=bqk_sb[:, oc:oc + 1])

    # v in natural token-major layout; each head padded to a 128-wide block
    # (col 64 = ones -> softmax denominator; cols 65..127 zero) so the ctx
    # matmul's stationary operand is a full 128-column bf16 tile (fast FWL).
    va_sb = []
    for sc in range(8):
        vt = va_pool.tile([128, H * 128], BF16, name=f"vaug{sc}", tag="vaug")
        va_sb.append(vt)
        vt_h = vt[:].rearrange("p (h c) -> p h c", c=128)
        nc.vector.memset(vt_h[:, :, HD + 1:], 0.0)
        nc.vector.tensor_copy(vt_h[:, :, HD:HD + 1],
                              ones_f32c[:].rearrange("p (h c) -> p h c", c=1))
        for off, width, h0 in ((0, 512, 0), (512, 256, 8)):
            nh = width // HD
            ps = pmm.tile([128, 512], F32, tag="ps")
            for kt in range(KT):
                nc.tensor.matmul(
                    ps[:, :width],
                    r(xt_sb[kt][:, sc * 128:(sc + 1) * 128]),
                    r(wv_sb[kt][:, off:off + width]),
                    start=(kt == 0), stop=(kt == KT - 1),
                )
            nc.vector.tensor_add(
                vt_h[:, h0:h0 + nh, 0:HD],
                ps[:, :width].rearrange("p (h c) -> p h c", c=HD),
                bvb_sb[:, off:off + width].rearrange("p (h c) -> p h c", c=HD),
            )
    p1_ctx.close()

    # ---------------- Phase 2: attention (per sequence, per head) ----------
    ctx_ctx = ExitStack()
    ctx_pool = ctx_ctx.enter_context(tc.tile_pool(name="ctxT", bufs=12, side="right"))
    ctx_sb = [ctx_pool.tile([128, 512], F32, name=f"ctx{i}", tag="ctx")
              for i in range(NSEQ * KT)]

    p2_ctx = ExitStack()
    pb_pool = p2_ctx.enter_context(tc.tile_pool(name="pbias", bufs=6, side="left"))
    sin_pool = p2_ctx.enter_context(tc.tile_pool(name="sin", bufs=6, side="left"))
    exp_pool = p2_ctx.enter_context(tc.tile_pool(name="exp", bufs=10, side="left"))
    rec_pool = p2_ctx.enter_context(tc.tile_pool(name="recip", bufs=2, side="left"))
    bcs_pool = p2_ctx.enter_context(tc.tile_pool(name="bcs", bufs=4, side="left"))

    for seq in range(NSEQ):
        for h in range(H):
            q_tile = qk_sb[h // 2]
            q_off = (h % 2) * 64
            k_tile = qk_sb[6 + h // 2]
            k_off = (h % 2) * 64
            # one batched DMA for all 4 key-chunks of this (seq, head)
            bt = pb_pool.tile([128, 4, 512], BF16, name=f"bt{seq}_{h}",
                              tag="bias")
            nc.gpsimd.dma_start(
                bt[:], expbT[seq, h].rearrange("(c p) i -> p c i", p=128))
            e_tiles = []
            for jt in range(4):
                ps = pmm.tile([128, 512], F32, tag="ps")
                nc.tensor.matmul(
                    ps[:],
                    k_tile[k_off:k_off + 64,
                           seq * 512 + jt * 128:seq * 512 + (jt + 1) * 128],
                    q_tile[q_off:q_off + 64, seq * 512:(seq + 1) * 512],
                    start=True, stop=True,
                )
                st = sin_pool.tile([128, 512], BF16, tag="sin")
                nc.scalar.activation(st[:], ps[:], AF.Exp)
                et = exp_pool.tile([128, 512], BF16, tag="exp")
                nc.vector.tensor_mul(et[:], st[:], bt[:, jt, :])
                e_tiles.append(et)

            pc = pctx.tile([128, 512], F32, tag="pctx")
            for jt in range(4):
                nc.tensor.matmul(
                    pc[:],
                    va_sb[seq * 4 + jt][:, h * 128:h * 128 + 128],
                    e_tiles[jt][:],
                    start=(jt == 0), stop=(jt == 3),
                )
            ld = rec_pool.tile([1, 512], F32, tag="ln")
            nc.scalar.activation(ld[:], pc[HD:HD + 1, :], AF.Ln)
            rc = rec_pool.tile([1, 512], F32, tag="recip")
            nc.scalar.activation(rc[:], ld[:], AF.Exp, scale=-1.0)
            bc = bcs_pool.tile([64, 512], F32, tag="bc")
            nc.gpsimd.partition_broadcast(bc[:], rc[:], channels=64)
            nc.vector.tensor_mul(
                ctx_sb[seq * KT + h // 2][(h % 2) * 64:(h % 2) * 64 + 64, :]
                .bitcast(F32R),
                pc[0:HD, :], bc[:])
    p2_ctx.close()
    qkva_ctx.close()

    # ---------------- Phase 3: Wo projection + residual -------------------
    s1_ctx = ExitStack()
    s1_pool = s1_ctx.enter_context(tc.tile_pool(name="s1", bufs=KT, side="left"))
    s1_sb = [s1_pool.tile([128, N], F32, name=f"s1_{oc}", tag="s1")
             for oc in range(KT)]

    p3_ctx = ExitStack()
    wo_pool = p3_ctx.enter_context(tc.tile_pool(name="wo", bufs=KT, side="left"))
    wo_sb = []
    for kt in range(KT):
        t = wo_pool.tile([128, DIM], F32R, name=f"wo{kt}", tag="wo")
        nc.sync.dma_start(t[:], woT[kt * 128:(kt + 1) * 128, :])
        wo_sb.append(t)

    for seq in range(NSEQ):
        for oc in range(KT):
            ps = pmm.tile([128, 512], F32, tag="ps")
            for kt in range(KT):
                nc.tensor.matmul(
                    ps[:],
                    r(wo_sb[kt][:, oc * 128:(oc + 1) * 128]),
                    r(ctx_sb[seq * KT + kt][:]),
                    start=(kt == 0), stop=(kt == KT - 1),
                )
            # s1 = wo_out + bo + x   (attention residual)
            nc.vector.scalar_tensor_tensor(
                s1_sb[oc][:, seq * 512:(seq + 1) * 512].bitcast(F32R),
                ps[:], bo_sb[:, oc:oc + 1],
                xt_sb[oc][:, seq * 512:(seq + 1) * 512].bitcast(F32),
                op0=OP.add, op1=OP.add,
            )
    p3_ctx.close()
    ctx_ctx.close()

    # ---------------- shared LayerNorm helper ------------------------------
    def layernorm(src_sb, dst_cb, sq_pool, stat_pool):
        """Feature-axis layernorm over KT source tiles [128, N]. Stats via
        ones-matmuls; mean/rstd broadcast across partitions by GPSIMD into
        SBUF; dst_cb(oc, half, mbc, rbc) applies."""
        for half in range(2):
            hs = slice(half * 512, (half + 1) * 512)
            psx = pstat.tile([1, 512], F32, tag="st")
            psxx = pstat.tile([1, 512], F32, tag="st")
            for oc in range(KT):
                sq = sq_pool.tile([128, 512], F32, tag="sq")
                nc.scalar.activation(sq[:].bitcast(F32R), src_sb[oc][:, hs],
                                     AF.Square)
                nc.tensor.matmul(psx[:], r(ones_col[:]), r(src_sb[oc][:, hs]),
                                 start=(oc == 0), stop=(oc == KT - 1))
                nc.tensor.matmul(psxx[:], r(ones_col[:]), r(sq[:]),
                                 start=(oc == 0), stop=(oc == KT - 1))
            m_sb = stat_pool.tile([1, 512], F32, tag="st")
            nc.scalar.activation(m_sb[:], psx[:], AF.Identity, scale=1.0 / DIM)
            msq = stat_pool.tile([1, 512], F32, tag="st")
            nc.scalar.activation(msq[:], psx[:], AF.Square, scale=1.0 / DIM)
            var = stat_pool.tile([1, 512], F32, tag="st")
            nc.vector.scalar_tensor_tensor(var[:], psxx[:], 1.0 / DIM, msq[:],
                                           op0=OP.mult, op1=OP.subtract)
            lv = stat_pool.tile([1, 512], F32, tag="st")
            nc.scalar.activation(lv[:], var[:], AF.Ln, bias=eps_sb[:1, :1])
            rs = stat_pool.tile([1, 512], F32, tag="st")
            nc.scalar.activation(rs[:], lv[:], AF.Exp, scale=-0.5)
            mbc = bcln_pool.tile([128, 512], F32, tag="mbc")
            nc.gpsimd.partition_broadcast(mbc[:], m_sb[:], channels=128)
            rbc = bcln_pool.tile([128, 512], F32, tag="rbc")
            nc.gpsimd.partition_broadcast(rbc[:], rs[:], channels=128)
            for oc in range(KT):
                dst_cb(oc, half, mbc, rbc)

    # ---------------- Phase 4: LayerNorm 1 --------------------------------
    rz_ctx = ExitStack()
    r_pool = rz_ctx.enter_context(tc.tile_pool(name="resid", bufs=KT, side="right"))
    r_sb = [r_pool.tile([128, N], F32, name=f"r{oc}", tag="resid")
            for oc in range(KT)]
    z1_ctx = ExitStack()
    z1_pool = z1_ctx.enter_context(tc.tile_pool(name="z1", bufs=KT, side="right"))
    z1_sb = [z1_pool.tile([128, N], F32, name=f"z1_{oc}", tag="z1")
             for oc in range(KT)]

    p4_ctx = ExitStack()
    sq1_pool = p4_ctx.enter_context(tc.tile_pool(name="sq1", bufs=4, side="left"))
    st1_pool = p4_ctx.enter_context(tc.tile_pool(name="st1", bufs=10, side="left"))
    t1_pool = p4_ctx.enter_context(tc.tile_pool(name="t1", bufs=3, side="left"))

    def ln1_apply(oc, half, mbc, rbc):
        hs = slice(half * 512, (half + 1) * 512)
        t = t1_pool.tile([128, 512], F32, tag="t")
        nc.vector.tensor_sub(t[:], s1_sb[oc][:, hs], mbc[:])
        nc.vector.tensor_mul(z1_sb[oc][:, hs].bitcast(F32R), t[:], rbc[:])

    layernorm(s1_sb, ln1_apply, sq1_pool, st1_pool)
    # residual carry: r = z1*g1 + (ln1_b + bdown)
    for oc in range(KT):
        nc.scalar.activation(r_sb[oc][:], z1_sb[oc][:], AF.Identity,
                             bias=c1_sb[:, oc:oc + 1], scale=g1_sb[:, oc:oc + 1])
    p4_ctx.close()
    s1_ctx.close()
    xt_ctx.close()

    # ---------------- Phase 5: GLU (gate = gelu(glu1+cb), mult branch) ----
    s2_ctx = ExitStack()
    s2_pool = s2_ctx.enter_context(tc.tile_pool(name="s2", bufs=KT, side="left"))
    s2_sb = [s2_pool.tile([128, N], F32, name=f"s2_{oc}", tag="s2")
             for oc in range(KT)]

    gated_ctx = ExitStack()
    gated_pool = gated_ctx.enter_context(
        tc.tile_pool(name="gated", bufs=24, side="left"))
    gated_sb = [gated_pool.tile([128, N], BF16, name=f"gated{j}", tag="gated")
                for j in range(24)]

    p5_ctx = ExitStack()
    wg_pool = p5_ctx.enter_context(tc.tile_pool(name="wglu", bufs=12, side="left"))
    gelu_pool = p5_ctx.enter_context(tc.tile_pool(name="gelu", bufs=4, side="left"))

    for g in range(3):
        wg1 = []
        wg2 = []
        for kt in range(KT):
            t = wg_pool.tile([128, 1024], F32R, name=f"wg1_{g}_{kt}", tag="wg")
            nc.sync.dma_start(
                t[:], wgluT[kt * 128:(kt + 1) * 128, g * 1024:(g + 1) * 1024])
            wg1.append(t)
            t2 = wg_pool.tile([128, 1024], F32R, name=f"wg2_{g}_{kt}", tag="wg")
            nc.sync.dma_start(
                t2[:], wgluT[kt * 128:(kt + 1) * 128,
                             I + g * 1024:I + (g + 1) * 1024])
            wg2.append(t2)
        for j in range(8):
            oc = g * 8 + j          # gate chunk index in [0, 24)
            for half in range(2):
                hs = slice(half * 512, (half + 1) * 512)
                ps = pmm.tile([128, 512], F32, tag="ps")
                for kt in range(KT):
                    nc.tensor.matmul(
                        ps[:], r(wg1[kt][:, j * 128:(j + 1) * 128]),
                        r(z1_sb[kt][:, hs]),
                        start=(kt == 0), stop=(kt == KT - 1))
                ge = gelu_pool.tile([128, 512], F32, tag="gelu")
                nc.scalar.activation(ge[:], ps[:], gelu_func,
                                     bias=cb1_sb[:, oc:oc + 1])
                ps2 = pmm.tile([128, 512], F32, tag="ps")
                for kt in range(KT):
                    nc.tensor.matmul(
                        ps2[:], r(wg2[kt][:, j * 128:(j + 1) * 128]),
                        r(z1_sb[kt][:, hs]),
                        start=(kt == 0), stop=(kt == KT - 1))
                # gated = (glu2 + cb1_2) * gelu(glu1 + cb1_1), stored bf16
                nc.vector.scalar_tensor_tensor(
                    gated_sb[oc][:, hs], ps2[:], cb1_sb[:, 24 + oc:25 + oc],
                    ge[:], op0=OP.add, op1=OP.mult)
    p5_ctx.close()
    z1_ctx.close()

    # ---------------- Phase 6: Wdown + residual ----------------------------
    p6_ctx = ExitStack()
    wd_pool = p6_ctx.enter_context(tc.tile_pool(name="wdown", bufs=24, side="left"))
    wd_sb = []
    for kt in range(24):
        t = wd_pool.tile([128, DIM], BF16, name=f"wd{kt}", tag="wd")
        nc.sync.dma_start(t[:], wdownT[kt * 128:(kt + 1) * 128, :])
        wd_sb.append(t)

    for half in range(2):
        hs = slice(half * 512, (half + 1) * 512)
        for oc in range(KT):
            ps = pmm.tile([128, 512], F32, tag="ps")
            for kt in range(24):
                nc.tensor.matmul(
                    ps[:], wd_sb[kt][:, oc * 128:(oc + 1) * 128],
                    gated_sb[kt][:, hs],
                    start=(kt == 0), stop=(kt == 23))
            nc.vector.tensor_add(s2_sb[oc][:, hs].bitcast(F32R), ps[:],
                                 r_sb[oc][:, hs])

    # ---------------- Phase 7: LayerNorm 2 + output ------------------------
    p7_ctx = ExitStack()
    out_pool = p7_ctx.enter_context(tc.tile_pool(name="outp", bufs=KT, side="left"))
    out_sb = [out_pool.tile([128, N], F32, name=f"out{oc}", tag="out")
              for oc in range(KT)]
    sq2_pool = p7_ctx.enter_context(tc.tile_pool(name="sq2", bufs=4, side="left"))
    st2_pool = p7_ctx.enter_context(tc.tile_pool(name="st2", bufs=10, side="left"))
    t2_pool = p7_ctx.enter_context(tc.tile_pool(name="t2", bufs=3, side="left"))

    def ln2_apply(oc, half, mbc, rbc):
        hs = slice(half * 512, (half + 1) * 512)
        t = t2_pool.tile([128, 512], F32, tag="t")
        nc.vector.tensor_sub(t[:], s2_sb[oc][:, hs], mbc[:])
        zz = t2_pool.tile([128, 512], F32, tag="zz")
        nc.vector.scalar_tensor_tensor(zz[:], t[:], g2_sb[:, oc:oc + 1],
                                       rbc[:], op0=OP.mult, op1=OP.mult)
        nc.scalar.activation(out_sb[oc][:, hs], zz[:], AF.Identity,
                             bias=b2_sb[:, oc:oc + 1])
        nc.sync.dma_start(outT[oc * 128:(oc + 1) * 128, hs], out_sb[oc][:, hs])

    layernorm(s2_sb, ln2_apply, sq2_pool, st2_pool)
    p7_ctx.close()
    p6_ctx.close()
    gated_ctx.close()
    rz_ctx.close()
    s2_ctx.close()
    root.close()


# ---------------------------------------------------------------------------
_NC_CACHE = None


def _get_nc():
    global _NC_CACHE
    if _NC_CACHE is None:
        _NC_CACHE = build_program()
    return _NC_CACHE


def _to128(v, cols):
    """(cols*128,) vector -> [128, cols] with column c = v[c*128:(c+1)*128]."""
    return np.ascontiguousarray(np.asarray(v, np.float32).reshape(cols, 128).T)


def prep_inputs(inputs):
    hs = np.asarray(inputs["hidden_states"], np.float32)
    bias = np.asarray(inputs["bias"], np.float32)
    Wqkv = np.asarray(inputs["Wqkv"], np.float32)
    bqkv = np.asarray(inputs["bqkv"], np.float32)
    Wo = np.asarray(inputs["Wo"], np.float32)
    bo_v = np.asarray(inputs["bo"], np.float32)
    ln1_g = np.asarray(inputs["ln1_g"], np.float32)
    ln1_b = np.asarray(inputs["ln1_b"], np.float32)
    Wglu = np.asarray(inputs["Wglu"], np.float32)
    Wdown = np.asarray(inputs["Wdown"], np.float32)
    bdown = np.asarray(inputs["bdown"], np.float32)
    ln2_g = np.asarray(inputs["ln2_g"], np.float32)
    ln2_b = np.asarray(inputs["ln2_b"], np.float32)

    x_t = np.ascontiguousarray(hs.T)                       # (768, 8192)
    expbT = np.exp(np.ascontiguousarray(bias.transpose(0, 1, 3, 2))
                   ).astype(ml_dtypes.bfloat16)

    scale = 1.0 / np.sqrt(np.float32(HD))
    Wq = Wqkv.copy()
    Wq[:DIM] *= scale                                      # fold 1/sqrt(hd) into q
    wqkvT = np.ascontiguousarray(Wq.T)                     # (768, 2304)
    bqk_v = bqkv[:2 * DIM].copy()
    bqk_v[:DIM] *= scale
    bv_b = np.ascontiguousarray(
        np.broadcast_to(bqkv[2 * DIM:], (128, DIM)).astype(np.float32))

    woT = np.ascontiguousarray(Wo.T)
    wgluT = np.ascontiguousarray((Wglu * ln1_g[None, :]).T)  # g1 folded
    cb1_v = Wglu @ ln1_b                                     # (6144,)
    c1_v = ln1_b + bdown
    wdownT = np.ascontiguousarray(Wdown.T).astype(ml_dtypes.bfloat16)

    shared = {
        "wqkvT": wqkvT,
        "bqk": _to128(bqk_v, 12),
        "bv_b": bv_b,
        "woT": woT,
        "bo": _to128(bo_v, 6),
        "wgluT": wgluT,
        "cb1": _to128(cb1_v, 48),
        "g1": _to128(ln1_g, 6),
        "c1": _to128(c1_v, 6),
        "wdownT": wdownT,
        "g2": _to128(ln2_g, 6),
        "b2": _to128(ln2_b, 6),
    }
    in_maps = []
    for c in range(N_CORES):
        m = dict(shared)
        m["xT"] = np.ascontiguousarray(x_t[:, c * N:(c + 1) * N])
        m["expbT"] = np.ascontiguousarray(expbT[c * NSEQ:(c + 1) * NSEQ])
        in_maps.append(m)
    return in_maps


def kernel(**inputs):
    nc = _get_nc()
    in_maps = prep_inputs(inputs)
    res = run_bass_kernel_spmd(nc, in_maps, core_ids=list(range(N_CORES)))
    outT = np.concatenate([res.results[c]["outT"] for c in range(N_CORES)],
                          axis=1)                          # (768, 8192)
    return np.ascontiguousarray(outT.T)
